# revision 15
# baseline (speedup 1.0000x reference)
"""Trainium2 Bass kernel for the single-query-attention diffusion decoder.

Full-input contract: kernel(**inputs) -> np.ndarray [B, V].
Data-parallel over batch across 8 NeuronCores (16 rows each).

Math (reference restructured):
    cond  = silu(pe[t] @ Wt1.T + bt1) @ Wt2.T + bt2            [B, D]
    q~    = (query + cond) @ M1,  M1 = Wq.T @ Wk               [B, D]
    s[v]  = q~ . T[v] + x[v]   (+ q~.cond, dropped: softmax shift-invariant)
    w     = softmax(s)
    ws    = sum_v w[v] T[v] + cond                             [D]
    base  = ws @ M3 + r0,  M3 = Wv.T @ Wp.T @ Wd1[:, :D].T,
            r0 = bp @ Wd1[:, :D].T + bd1
    p[v]  = sum_j w2[j] relu(T[v] @ Bm + base)[j] + bd2 + w[v]
            Bm = Wd1[:, D:].T,  w2 = Wd2[0]

Decoder sign trick: with |w2| folded into Bm/M3/r0 (columns scaled) and the
j axis permuted so sign(w2) = +1 columns come first (jp kept even so the DVE
slices stay 4B-aligned),
    w2[j] relu(h + base)[j] = sgn_j * (max(h^, -base^) + base^)_j
so p[v] = [sum_{j<jp} - sum_{j>=jp}] max(h^_jv, -base^_j)  + C + bd2 + w[v]
with C = sum_j sgn_j base^_j.  Each H PSUM tile is drained by two fused
tensor_tensor_reduce ops (max + add-reduce in one DVE instruction) -- no PE
base-fold matmuls, no separate accumulator reads.  base is one batched PE
matmul per row group; -base^ bounces through DRAM into an fp16
partition-broadcast SBUF tile.  ppos/pneg/negbase/w are DMA'd out and the
final p (+C +bd2 +w) is assembled on host.
"""

import os
import sys

for _p in ("/opt/trn_rl_repo", "/opt/trn_rl_repo/concourse"):
    if os.path.isdir(_p) and _p not in sys.path:
        sys.path.append(_p)

import numpy as np
import ml_dtypes

import concourse.bass as bass
import concourse.tile as tile
from concourse import bacc, mybir
from concourse.bass_utils import run_bass_kernel_spmd

F32 = mybir.dt.float32
F16 = mybir.dt.float16
BF16 = mybir.dt.bfloat16
I32 = mybir.dt.int32
AF = mybir.ActivationFunctionType
ALU = mybir.AluOpType
BF_NP = ml_dtypes.bfloat16

NCORES = 8
B = 128
BSH = B // NCORES  # 16 batch rows per core
D = 512
V = 1024
J = 2 * D  # 1024 decoder hidden
DC = D // 128  # 4 d-chunks
VT = V // 128  # 8 v-tiles
MAX_LEN = 5000

# base-matmul row groups: pairs, emitted at iteration 2k+1 right after the
# interleaved ws chunks of row 2k+1 complete; decode(b) runs at iteration
# b+2 so its group (emitted at iteration b|1 <= b+1) is always ready.
GROUPS = [(2 * k, 2 * k + 1) for k in range(BSH // 2)]
# iteration -> decoded row (attention row i is emitted at iteration i)
DEC_AT = {i: i - 2 for i in range(2, 18)}
N_ITER = 18


def build_nc(jp: int) -> bass.Bass:
    # Bacc (not plain Bass): its finalize() legalizes sync waits
    # (generate_event_semaphores) to TRN2's 1-wait-per-instruction limit.
    nc = bacc.Bacc()

    # ---- per-core inputs ----
    te_d = nc.declare_dram_parameter("te", [BSH, D, V], BF16, isOutput=False)
    x_d = nc.declare_dram_parameter("x", [BSH, V], BF16, isOutput=False)
    ts_d = nc.declare_dram_parameter("ts", [BSH, 1], I32, isOutput=False)
    qet_d = nc.declare_dram_parameter("qet", [D, BSH], BF16, isOutput=False)
    # ---- replicated (host-folded) weights ----
    pe_d = nc.declare_dram_parameter("pe", [MAX_LEN, D], F32, isOutput=False)
    wt1t_d = nc.declare_dram_parameter("wt1t", [D, D], BF16, isOutput=False)
    wt2t_d = nc.declare_dram_parameter("wt2t", [D, D], BF16, isOutput=False)
    bt1c_d = nc.declare_dram_parameter("bt1c", [128, DC], F32, isOutput=False)
    bt2c_d = nc.declare_dram_parameter("bt2c", [128, DC], F32, isOutput=False)
    m1_d = nc.declare_dram_parameter("m1", [D, D], BF16, isOutput=False)
    m3_d = nc.declare_dram_parameter("m3", [D, J], BF16, isOutput=False)
    bm_d = nc.declare_dram_parameter("bm", [D, J], BF16, isOutput=False)
    r0_d = nc.declare_dram_parameter("r0", [J], BF16, isOutput=False)
    # ---- outputs (host assembles p from these) ----
    nb_d = nc.declare_dram_parameter("nb", [BSH, J], F32, isOutput=True)
    wout_d = nc.declare_dram_parameter("wout", [BSH, V], BF16, isOutput=True)
    pp_d = nc.declare_dram_parameter("pp", [BSH, 2, 128, VT], F32, isOutput=True)

    with tile.TileContext(nc) as tc:
        with (
            tc.tile_pool(name="w", bufs=1) as wp,
            tc.tile_pool(name="te", bufs=BSH) as tep,
            tc.tile_pool(name="rows", bufs=3) as rowp,
            tc.tile_pool(name="nbc", bufs=2) as nbp,
            tc.tile_pool(name="ebc", bufs=2) as ebp,
            tc.tile_pool(name="scr", bufs=1) as scrp,
            tc.tile_pool(name="tiny", bufs=8) as tinyp,
            tc.tile_pool(name="dramp", bufs=1, space="DRAM") as dramp,
            tc.tile_pool(name="hp", bufs=4, space="PSUM") as hp,
        ):
            st = [dict() for _ in range(BSH)]

            def emit_loads(b):
                s = st[b]
                if "te" in s:
                    return
                s["xrow"] = rowp.tile([1, V], BF16, tag="xrow", name=f"xrow{b}")
                nc.sync.dma_start(out=s["xrow"], in_=x_d[b:b + 1, :])
                s["te"] = tep.tile([128, DC, V], BF16, tag="te", name=f"te{b}")
                nc.sync.dma_start(
                    out=s["te"], in_=te_d[b].rearrange("(c p) v -> p c v", p=128)
                )

            # ====== loads, ordered so the pipeline starts ASAP ======
            ts_sb = wp.tile([BSH, 1], I32, tag="ts")
            nc.sync.dma_start(out=ts_sb, in_=ts_d[:])
            # dummy 1-row gather warms the Q7 indirect-DMA kernel (~6us IRAM
            # load) while the ts/weight DMAs stream in
            zidx = wp.tile([2, 1], I32, tag="zidx")
            nc.vector.memset(zidx, 0)
            warm_g = wp.tile([2, D], F32, tag="warm_g")
            nc.gpsimd.indirect_dma_start(
                out=warm_g[:],
                out_offset=None,
                in_=pe_d[:],
                in_offset=bass.IndirectOffsetOnAxis(ap=zidx[:, :1], axis=0),
            )
            # gather pe rows by timestep (gpsimd queue; only needs ts)
            tpe = wp.tile([BSH, D], F32, tag="tpe")
            nc.gpsimd.indirect_dma_start(
                out=tpe[:],
                out_offset=None,
                in_=pe_d[:],
                in_offset=bass.IndirectOffsetOnAxis(ap=ts_sb[:, :1], axis=0),
            )
            wt1t = wp.tile([128, DC, D], BF16, tag="wt1t")
            nc.sync.dma_start(out=wt1t, in_=wt1t_d[:].rearrange("(c p) z -> p c z", p=128))
            emit_loads(0)
            bt1c = wp.tile([128, DC], F32, tag="bt1c")
            nc.sync.dma_start(out=bt1c, in_=bt1c_d[:])
            bt2c = wp.tile([128, DC], F32, tag="bt2c")
            nc.sync.dma_start(out=bt2c, in_=bt2c_d[:])
            wt2t = wp.tile([128, DC, D], BF16, tag="wt2t")
            nc.sync.dma_start(out=wt2t, in_=wt2t_d[:].rearrange("(c p) z -> p c z", p=128))
            emit_loads(1)
            qet = wp.tile([128, DC, BSH], BF16, tag="qet")
            nc.sync.dma_start(out=qet, in_=qet_d[:].rearrange("(c p) b -> p c b", p=128))
            m1 = wp.tile([128, DC, D], BF16, tag="m1")
            nc.sync.dma_start(out=m1, in_=m1_d[:].rearrange("(c p) z -> p c z", p=128))
            emit_loads(2)
            m3 = wp.tile([128, DC, J], BF16, tag="m3")
            nc.sync.dma_start(out=m3, in_=m3_d[:].rearrange("(c p) j -> p c j", p=128))
            bm = wp.tile([128, DC, J], BF16, tag="bm")
            nc.sync.dma_start(out=bm, in_=bm_d[:].rearrange("(c p) j -> p c j", p=128))
            emit_loads(3)
            # r0 staged on partition row 0 (rhs of K=1 fold matmuls)
            r01 = wp.tile([1, J], BF16, tag="r01")
            nc.sync.dma_start(
                out=r01, in_=bass.AP(tensor=r0_d, offset=0, ap=[[J, 1], [1, J]])
            )
            ones1 = wp.tile([1, 128], BF16, tag="ones1")
            nc.vector.memset(ones1, 1.0)
            id128 = wp.tile([128, 128], F32, tag="id128")
            from concourse.masks import make_identity

            make_identity(nc, id128)
            # bf16 identity produced by ACT (keeps transpose waits mergeable)
            id_bf = wp.tile([BSH, BSH], BF16, tag="id_bf")
            nc.scalar.activation(out=id_bf, in_=id128[:BSH, :BSH], func=AF.Copy)
            # PE warmup on id128 so later fp32 transposes never owe a Pool wait
            warm_ps = hp.tile([2, 2], F32, tag="h")
            nc.tensor.transpose(warm_ps, id128[0:2, 0:2], id128[0:2, 0:2])
            # fp16 -base^ rows staged in DRAM for the partition broadcast
            nb16_t = dramp.tile([BSH, J], F16, tag="nb16")

            # ================= setup: cond / q~ =================
            tpe_bf = wp.tile([BSH, D], BF16, tag="tpe_bf")
            nc.scalar.activation(out=tpe_bf, in_=tpe, func=AF.Copy)
            tpeT = wp.tile([128, DC, BSH], BF16, tag="tpeT")
            for c in range(DC):
                ps = hp.tile([128, BSH], BF16, tag="h")
                nc.tensor.transpose(ps, tpe_bf[:, c * 128:(c + 1) * 128], id_bf)
                nc.scalar.activation(out=tpeT[:, c, :], in_=ps, func=AF.Copy)
            # Z.T = Wt1 @ tpe.T (+bt1), silu = z * sigmoid(z)
            s_sb = wp.tile([128, DC, BSH], BF16, tag="s_sb")
            zl_sb = wp.tile([128, DC, BSH], F32, tag="zl_sb")
            sg_sb = wp.tile([128, DC, BSH], F32, tag="sg_sb")
            for zt in range(DC):
                ps = hp.tile([128, BSH], F32, tag="h")
                for c in range(DC):
                    nc.tensor.matmul(
                        ps, wt1t[:, c, zt * 128:(zt + 1) * 128], tpeT[:, c, :],
                        start=(c == 0), stop=(c == DC - 1),
                    )
                nc.scalar.activation(
                    out=zl_sb[:, zt, :], in_=ps, func=AF.Identity,
                    bias=bt1c[:, zt:zt + 1], scale=1.0,
                )
                nc.scalar.activation(
                    out=sg_sb[:, zt, :], in_=ps, func=AF.Sigmoid,
                    bias=bt1c[:, zt:zt + 1], scale=1.0,
                )
            nc.vector.tensor_mul(
                s_sb.rearrange("p c b -> p (c b)"),
                zl_sb.rearrange("p c b -> p (c b)"),
                sg_sb.rearrange("p c b -> p (c b)"),
            )
            # condT = Wt2 @ silu (+bt2)
            condT = wp.tile([128, DC, BSH], BF16, tag="condT")
            for ct in range(DC):
                ps = hp.tile([128, BSH], F32, tag="h")
                for c in range(DC):
                    nc.tensor.matmul(
                        ps, wt2t[:, c, ct * 128:(ct + 1) * 128], s_sb[:, c, :],
                        start=(c == 0), stop=(c == DC - 1),
                    )
                nc.scalar.activation(
                    out=condT[:, ct, :], in_=ps, func=AF.Identity,
                    bias=bt2c[:, ct:ct + 1], scale=1.0,
                )
            # qcT = qeT + condT ; q~T = M1.T @ qcT  (bf16)
            qcT = wp.tile([128, DC, BSH], BF16, tag="qcT")
            nc.vector.tensor_add(qcT[:], qet[:], condT[:])
            qtT = wp.tile([128, DC, BSH], BF16, tag="qtT")
            for mt in range(DC):
                ps = hp.tile([128, BSH], F32, tag="h")
                for c in range(DC):
                    nc.tensor.matmul(
                        ps, m1[:, c, mt * 128:(mt + 1) * 128], qcT[:, c, :],
                        start=(c == 0), stop=(c == DC - 1),
                    )
                nc.scalar.activation(out=qtT[:, mt, :], in_=ps, func=AF.Copy)

            # ws across all rows (read by batched base matmuls)
            ws_sb = wp.tile([128, DC, BSH], BF16, tag="ws_sb")

            # ============ skewed pipeline over batch rows ============
            def emit_attn(b):
                """scores (PE) -> exp/norm (ACT) -> w bounce."""
                s = st[b]
                emit_loads(b)
                te_t, xrow = s["te"], s["xrow"]
                scs = hp.tile([1, 2, 512], F32, tag="h", name=f"sc{b}")
                sc = [scs[:, 0, :], scs[:, 1, :]]
                for h in range(2):
                    for c in range(DC):
                        nc.tensor.matmul(
                            sc[h], qtT[:, c, b:b + 1],
                            te_t[:, c, h * 512:(h + 1) * 512],
                            start=(c == 0), stop=False,
                        )
                for h in range(2):
                    nc.tensor.matmul(
                        sc[h], ones1[0:1, 0:1],
                        xrow[0:1, h * 512:(h + 1) * 512],
                        start=False, stop=True,
                    )
                exp_row = rowp.tile([1, V], F32, tag="exp", name=f"exp{b}")
                se = [tinyp.tile([1, 1], F32, tag="t1", name=f"se{h}_{b}") for h in range(2)]
                for h in range(2):
                    nc.scalar.activation(
                        out=exp_row[:, h * 512:(h + 1) * 512], in_=sc[h],
                        func=AF.Exp, accum_out=se[h],
                    )
                sume = tinyp.tile([1, 1], F32, tag="t1", name=f"sume{b}")
                nc.vector.tensor_add(sume, se[0], se[1])
                rec = tinyp.tile([1, 1], F32, tag="t1", name=f"rec{b}")
                nc.vector.reciprocal(rec, sume)
                expn = rowp.tile([1, V], BF16, tag="expn", name=f"expn{b}")
                nc.scalar.activation(
                    out=expn, in_=exp_row, func=AF.Copy, bias=0.0, scale=rec[:, :1]
                )
                # w row out (also the DRAM bounce source for the broadcast)
                nc.sync.dma_start(out=wout_d[b:b + 1, :], in_=expn)
                ebc = ebp.tile([128, V], BF16, tag="ebc", name=f"ebc{b}")
                nc.sync.dma_start(
                    out=ebc,
                    in_=bass.AP(tensor=wout_d, offset=b * V, ap=[[0, 128], [1, V]]),
                )
                s["ebc"] = ebc

            def ws_chunk(b, c):
                """one DVE multiply+reduce chunk of ws row b; c==DC adds cond."""
                s = st[b]
                if c == DC:
                    nc.vector.tensor_add(
                        ws_sb[:, :, b:b + 1].rearrange("p c one -> p (c one)"),
                        s["ws2"], condT[:, :, b],
                    )
                    return
                if "ws2" not in s:
                    s["ws2"] = tinyp.tile([128, DC], F32, tag="ws2", name=f"ws2_{b}")
                wscr = scrp.tile([128, V], BF16, tag="wscr")
                nc.vector.scalar_tensor_tensor(
                    out=wscr, in0=s["te"][:, c, :], scalar=0.0, in1=s["ebc"],
                    op0=ALU.bypass, op1=ALU.mult,
                    accum_out=s["ws2"][:, c:c + 1],
                )

            def emit_ws(b):
                for c in range(DC + 1):
                    ws_chunk(b, c)

            def emit_base(lo, hi):
                """batched base matmul for rows lo..hi; -base^ -> DRAM."""
                n = hi - lo + 1
                bp_ps = hp.tile([4, J], F32, tag="h", name=f"base{lo}")
                for h in range(2):
                    for c in range(DC):
                        nc.tensor.matmul(
                            bp_ps[:n, h * 512:(h + 1) * 512],
                            ws_sb[:, c, lo:hi + 1],
                            m3[:, c, h * 512:(h + 1) * 512],
                            start=(c == 0), stop=False,
                        )
                for h in range(2):
                    nc.tensor.matmul(
                        bp_ps[:n, h * 512:(h + 1) * 512],
                        ones1[0:1, 0:n], r01[0:1, h * 512:(h + 1) * 512],
                        start=False, stop=True,
                    )
                negb16 = wp.tile([4, J], F16, tag="negb16", name=f"negb16_{lo}")
                nc.scalar.activation(
                    out=negb16[:n], in_=bp_ps[:n], func=AF.Copy, bias=0.0, scale=-1.0
                )
                nc.sync.dma_start(out=nb16_t[lo:hi + 1, :], in_=negb16[:n])
                negb = wp.tile([4, J], F32, tag="negb", name=f"negb{lo}")
                nc.scalar.activation(
                    out=negb[:n], in_=bp_ps[:n], func=AF.Copy, bias=0.0, scale=-1.0
                )
                nc.sync.dma_start(out=nb_d[lo:hi + 1, :], in_=negb[:n])

            def emit_decode(b, ws_row=None):
                """H matmuls (PE) + sign-split max/accum drains (DVE), with the
                ws chunks of row ws_row threaded between drain pairs so they
                never block a full row of drains."""
                s = st[b]
                te_t = s["te"]
                nbc = nbp.tile([128, J], F16, tag="nbc", name=f"nbc{b}")
                nc.sync.dma_start(
                    out=nbc,
                    in_=bass.AP(tensor=nb16_t.tensor, offset=nb16_t.offset + b * J,
                                ap=[[0, 128], [1, J]]),
                )
                ppos = tinyp.tile([128, VT], F32, tag="ppos", name=f"ppos{b}")
                pneg = tinyp.tile([128, VT], F32, tag="pneg", name=f"pneg{b}")
                for vt in range(VT):
                    t = hp.tile([128, 2, 512], F32, tag="h", name=f"h{b}_{vt}")
                    for c in range(DC):
                        for h in range(2):
                            nc.tensor.matmul(
                                t[:, h, :],
                                te_t[:, c, vt * 128:(vt + 1) * 128],
                                bm[:, c, h * 512:(h + 1) * 512],
                                start=(c == 0), stop=(c == DC - 1),
                            )
                    tf = t.rearrange("p a v -> p (a v)")
                    nc.vector.scalar_tensor_tensor(
                        out=tf[:, :jp], in0=tf[:, :jp], scalar=0.0,
                        in1=nbc[:, :jp], op0=ALU.bypass, op1=ALU.max,
                        accum_out=ppos[:, vt:vt + 1],
                    )
                    nc.vector.scalar_tensor_tensor(
                        out=tf[:, jp:], in0=tf[:, jp:], scalar=0.0,
                        in1=nbc[:, jp:], op0=ALU.bypass, op1=ALU.max,
                        accum_out=pneg[:, vt:vt + 1],
                    )
                    if ws_row is not None and vt % 2 == 1:
                        ws_chunk(ws_row, vt // 2)
                        if vt == VT - 1:
                            ws_chunk(ws_row, DC)
                nc.sync.dma_start(out=pp_d[b, 0], in_=ppos)
                nc.sync.dma_start(out=pp_d[b, 1], in_=pneg)

            for i in range(N_ITER):
                if i + 2 < BSH:
                    emit_loads(i + 2)
                if i < BSH:
                    emit_attn(i)
                b = DEC_AT.get(i)
                if b is None:
                    if i < BSH:
                        emit_ws(i)
                else:
                    emit_decode(b, ws_row=(i if i < BSH else None))
                    st[b].clear()
                for (lo, hi) in GROUPS:
                    if hi == i:
                        emit_base(lo, hi)

    return nc


_NC_CACHE: dict = {}


def _get_nc(jp: int) -> bass.Bass:
    if jp not in _NC_CACHE:
        nc = build_nc(jp)
        nc.finalize()
        _NC_CACHE[jp] = nc
    return _NC_CACHE[jp]


def _pos_encoding() -> np.ndarray:
    pos = np.arange(MAX_LEN, dtype=np.float32)[:, None]
    div = np.exp(np.arange(0, D, 2, dtype=np.float32) * (-np.log(10000.0) / D))
    pe = np.zeros((MAX_LEN, D), dtype=np.float32)
    pe[:, 0::2] = np.sin(pos * div)
    pe[:, 1::2] = np.cos(pos * div)
    return pe


def prepare_in_maps(inputs: dict):
    f32 = lambda a: np.ascontiguousarray(np.asarray(a), dtype=np.float32)
    bf = lambda a: np.ascontiguousarray(np.asarray(a, dtype=np.float32).astype(BF_NP))
    x = np.asarray(inputs["x"], dtype=np.float32)
    ts = np.ascontiguousarray(np.asarray(inputs["timesteps"]).astype(np.int32).reshape(B, 1))
    qe = np.asarray(inputs["query_emb"], dtype=np.float32)
    te = np.asarray(inputs["target_emb"], dtype=np.float32)
    Wq, Wk, Wv, Wp = (f32(inputs[k]) for k in ("Wq", "Wk", "Wv", "Wp"))
    bp = f32(inputs["bp"])
    Wt1, bt1, Wt2, bt2 = (f32(inputs[k]) for k in ("Wt1", "bt1", "Wt2", "bt2"))
    Wd1, bd1, Wd2, bd2 = (f32(inputs[k]) for k in ("Wd1", "bd1", "Wd2", "bd2"))

    pe = _pos_encoding()
    M1 = Wq.T @ Wk
    A = np.ascontiguousarray(Wd1[:, :D].T)
    Bm = Wd1[:, D:].T
    M3 = (Wv.T @ Wp.T) @ A
    r0 = bp @ A + bd1
    w2 = Wd2[0].copy()
    bd2_val = float(bd2.reshape(-1)[0])
    bt1c = np.ascontiguousarray(bt1.reshape(DC, 128).T)
    bt2c = np.ascontiguousarray(bt2.reshape(DC, 128).T)

    # sign permutation: positive-w2 columns first; |w2| folded into the
    # j-indexed tensors so the decoder reduction is a plain +/- sum.  jp is
    # kept even (DVE 2x alignment) by zeroing the smallest-|w2| positive
    # column and placing it in the negative block (contributes exactly 0;
    # its true contribution is ~|w2|min * 0.5 ~ 1e-5, far below tolerance).
    pos_idx = np.where(w2 >= 0)[0]
    neg_idx = np.where(w2 < 0)[0]
    if len(pos_idx) % 2 == 1:
        drop = pos_idx[np.argmin(np.abs(w2[pos_idx]))]
        w2[drop] = 0.0
        pos_idx = pos_idx[pos_idx != drop]
        neg_idx = np.concatenate([neg_idx, [drop]])
    perm = np.concatenate([pos_idx, neg_idx])
    jp = int(len(pos_idx))
    aw = np.abs(w2)[perm]
    Bmh = np.ascontiguousarray(Bm[:, perm] * aw)
    M3h = np.ascontiguousarray(M3[:, perm] * aw)
    r0h = np.ascontiguousarray(r0[perm] * aw)

    shared = dict(
        pe=pe, wt1t=bf(Wt1.T), wt2t=bf(Wt2.T), bt1c=bt1c, bt2c=bt2c,
        m1=bf(M1), m3=bf(M3h), bm=bf(Bmh), r0=bf(r0h),
    )
    in_maps = []
    for i in range(NCORES):
        s = slice(i * BSH, (i + 1) * BSH)
        in_maps.append(
            dict(
                te=bf(te[s].transpose(0, 2, 1)),
                x=bf(x[s]),
                ts=np.ascontiguousarray(ts[s]),
                qet=bf(qe[s].T),
                **shared,
            )
        )
    return in_maps, jp, bd2_val


def assemble(results: list, jp: int, bd2_val: float) -> np.ndarray:
    """Host-side final assembly from per-core device outputs."""
    outs = []
    for r in results:
        pp = np.asarray(r["pp"], dtype=np.float32).reshape(BSH, 2, 128, VT)
        nb = np.asarray(r["nb"], dtype=np.float32).reshape(BSH, J)  # -base^
        w = np.asarray(r["wout"]).astype(np.float32).reshape(BSH, V)
        C = -(nb[:, :jp].sum(axis=1) - nb[:, jp:].sum(axis=1))  # [BSH]
        pc = pp[:, 0] - pp[:, 1]                                # [BSH, 128, VT]
        p = pc.transpose(0, 2, 1).reshape(BSH, V)               # v = vt*128 + part
        outs.append(p + C[:, None] + bd2_val + w)
    return np.concatenate(outs, axis=0).astype(np.float32)


def run(inputs: dict, trace: bool = False):
    in_maps, jp, bd2_val = prepare_in_maps(inputs)
    nc = _get_nc(jp)
    res = run_bass_kernel_spmd(nc, in_maps, list(range(NCORES)), trace=trace)
    out = assemble(res.results, jp, bd2_val)
    return out, res


def kernel(**inputs) -> np.ndarray:
    out, _ = run(inputs, trace=False)
    return out


# revision 16
# speedup vs baseline: 1.0490x; 1.0490x over previous
"""Trainium2 Bass kernel for the single-query-attention diffusion decoder.

Full-input contract: kernel(**inputs) -> np.ndarray [B, V].
Data-parallel over batch across 8 NeuronCores (16 rows each).

Math (reference restructured):
    cond  = silu(pe[t] @ Wt1.T + bt1) @ Wt2.T + bt2            [B, D]
    q~    = (query + cond) @ M1,  M1 = Wq.T @ Wk               [B, D]
    s[v]  = q~ . T[v] + x[v]   (+ q~.cond, dropped: softmax shift-invariant)
    w     = softmax(s)
    ws    = sum_v w[v] T[v] + cond                             [D]
    base  = ws @ M3 + r0,  M3 = Wv.T @ Wp.T @ Wd1[:, :D].T,
            r0 = bp @ Wd1[:, :D].T + bd1
    p[v]  = sum_j w2[j] relu(T[v] @ Bm + base)[j] + bd2 + w[v]
            Bm = Wd1[:, D:].T,  w2 = Wd2[0]

Decoder sign trick: with |w2| folded into Bm/M3/r0 (columns scaled) and the
j axis permuted so sign(w2) = +1 columns come first (jp kept even so the DVE
slices stay 4B-aligned),
    w2[j] relu(h + base)[j] = sgn_j * (max(h^, -base^) + base^)_j
so p[v] = [sum_{j<jp} - sum_{j>=jp}] max(h^_jv, -base^_j)  + C + bd2 + w[v]
with C = sum_j sgn_j base^_j.  Each H PSUM tile is drained by two fused
tensor_tensor_reduce ops (max + add-reduce in one DVE instruction) -- no PE
base-fold matmuls, no separate accumulator reads.  base is one batched PE
matmul per row group; -base^ bounces through DRAM into an fp16
partition-broadcast SBUF tile.  ppos/pneg/negbase/w are DMA'd out and the
final p (+C +bd2 +w) is assembled on host.
"""

import os
import sys

for _p in ("/opt/trn_rl_repo", "/opt/trn_rl_repo/concourse"):
    if os.path.isdir(_p) and _p not in sys.path:
        sys.path.append(_p)

import numpy as np
import ml_dtypes

import concourse.bass as bass
import concourse.tile as tile
from concourse import bacc, mybir
from concourse.bass_utils import run_bass_kernel_spmd

F32 = mybir.dt.float32
F16 = mybir.dt.float16
BF16 = mybir.dt.bfloat16
I32 = mybir.dt.int32
AF = mybir.ActivationFunctionType
ALU = mybir.AluOpType
BF_NP = ml_dtypes.bfloat16

NCORES = 8
B = 128
BSH = B // NCORES  # 16 batch rows per core
D = 512
V = 1024
J = 2 * D  # 1024 decoder hidden
DC = D // 128  # 4 d-chunks
VT = V // 128  # 8 v-tiles
MAX_LEN = 5000

# base-matmul row groups: pairs, emitted at iteration 2k+1 right after the
# interleaved ws chunks of row 2k+1 complete; decode(b) runs at iteration
# b+2 so its group (emitted at iteration b|1 <= b+1) is always ready.
GROUPS = [(2 * k, 2 * k + 1) for k in range(BSH // 2)]
# iteration -> decoded row (attention row i is emitted at iteration i)
DEC_AT = {i: i - 2 for i in range(2, 18)}
N_ITER = 18


def build_nc(jp: int) -> bass.Bass:
    # Bacc (not plain Bass): its finalize() legalizes sync waits
    # (generate_event_semaphores) to TRN2's 1-wait-per-instruction limit.
    nc = bacc.Bacc()

    # ---- per-core inputs ----
    te_d = nc.declare_dram_parameter("te", [BSH, D, V], BF16, isOutput=False)
    x_d = nc.declare_dram_parameter("x", [BSH, V], BF16, isOutput=False)
    ts_d = nc.declare_dram_parameter("ts", [BSH, 1], I32, isOutput=False)
    qet_d = nc.declare_dram_parameter("qet", [D, BSH], BF16, isOutput=False)
    # ---- replicated (host-folded) weights ----
    pe_d = nc.declare_dram_parameter("pe", [MAX_LEN, D], F32, isOutput=False)
    wt1t_d = nc.declare_dram_parameter("wt1t", [D, D], BF16, isOutput=False)
    wt2t_d = nc.declare_dram_parameter("wt2t", [D, D], BF16, isOutput=False)
    bt1c_d = nc.declare_dram_parameter("bt1c", [128, DC], F32, isOutput=False)
    bt2c_d = nc.declare_dram_parameter("bt2c", [128, DC], F32, isOutput=False)
    m1_d = nc.declare_dram_parameter("m1", [D, D], BF16, isOutput=False)
    m3_d = nc.declare_dram_parameter("m3", [D, J], BF16, isOutput=False)
    bm_d = nc.declare_dram_parameter("bm", [D, J], BF16, isOutput=False)
    r0_d = nc.declare_dram_parameter("r0", [J], BF16, isOutput=False)
    # ---- outputs (host assembles p from these) ----
    nb_d = nc.declare_dram_parameter("nb", [BSH, J], F32, isOutput=True)
    wout_d = nc.declare_dram_parameter("wout", [BSH, V], BF16, isOutput=True)
    pp_d = nc.declare_dram_parameter("pp", [BSH, 2, 128, VT], F32, isOutput=True)

    with tile.TileContext(nc) as tc:
        with (
            tc.tile_pool(name="w", bufs=1) as wp,
            tc.tile_pool(name="te", bufs=BSH) as tep,
            tc.tile_pool(name="rows", bufs=3) as rowp,
            tc.tile_pool(name="nbc", bufs=2) as nbp,
            tc.tile_pool(name="ebc", bufs=2) as ebp,
            tc.tile_pool(name="scr", bufs=1) as scrp,
            tc.tile_pool(name="tiny", bufs=8) as tinyp,
            tc.tile_pool(name="dramp", bufs=1, space="DRAM") as dramp,
            tc.tile_pool(name="hp", bufs=4, space="PSUM") as hp,
        ):
            st = [dict() for _ in range(BSH)]

            def emit_loads(b):
                s = st[b]
                if "te" in s:
                    return
                s["xrow"] = rowp.tile([1, V], BF16, tag="xrow", name=f"xrow{b}")
                nc.sync.dma_start(out=s["xrow"], in_=x_d[b:b + 1, :])
                s["te"] = tep.tile([128, DC, V], BF16, tag="te", name=f"te{b}")
                nc.sync.dma_start(
                    out=s["te"], in_=te_d[b].rearrange("(c p) v -> p c v", p=128)
                )

            # ====== loads, ordered so the pipeline starts ASAP ======
            ts_sb = wp.tile([BSH, 1], I32, tag="ts")
            nc.sync.dma_start(out=ts_sb, in_=ts_d[:])
            # dummy 1-row gather warms the Q7 indirect-DMA kernel (~6us IRAM
            # load) while the ts/weight DMAs stream in
            zidx = wp.tile([2, 1], I32, tag="zidx")
            nc.vector.memset(zidx, 0)
            warm_g = wp.tile([2, D], F32, tag="warm_g")
            nc.gpsimd.indirect_dma_start(
                out=warm_g[:],
                out_offset=None,
                in_=pe_d[:],
                in_offset=bass.IndirectOffsetOnAxis(ap=zidx[:, :1], axis=0),
            )
            # gather pe rows by timestep (gpsimd queue; only needs ts)
            tpe = wp.tile([BSH, D], F32, tag="tpe")
            nc.gpsimd.indirect_dma_start(
                out=tpe[:],
                out_offset=None,
                in_=pe_d[:],
                in_offset=bass.IndirectOffsetOnAxis(ap=ts_sb[:, :1], axis=0),
            )
            wt1t = wp.tile([128, DC, D], BF16, tag="wt1t")
            nc.sync.dma_start(out=wt1t, in_=wt1t_d[:].rearrange("(c p) z -> p c z", p=128))
            emit_loads(0)
            bt1c = wp.tile([128, DC], F32, tag="bt1c")
            nc.sync.dma_start(out=bt1c, in_=bt1c_d[:])
            bt2c = wp.tile([128, DC], F32, tag="bt2c")
            nc.sync.dma_start(out=bt2c, in_=bt2c_d[:])
            wt2t = wp.tile([128, DC, D], BF16, tag="wt2t")
            nc.sync.dma_start(out=wt2t, in_=wt2t_d[:].rearrange("(c p) z -> p c z", p=128))
            emit_loads(1)
            qet = wp.tile([128, DC, BSH], BF16, tag="qet")
            nc.sync.dma_start(out=qet, in_=qet_d[:].rearrange("(c p) b -> p c b", p=128))
            m1 = wp.tile([128, DC, D], BF16, tag="m1")
            nc.sync.dma_start(out=m1, in_=m1_d[:].rearrange("(c p) z -> p c z", p=128))
            emit_loads(2)
            m3 = wp.tile([128, DC, J], BF16, tag="m3")
            nc.sync.dma_start(out=m3, in_=m3_d[:].rearrange("(c p) j -> p c j", p=128))
            bm = wp.tile([128, DC, J], BF16, tag="bm")
            nc.sync.dma_start(out=bm, in_=bm_d[:].rearrange("(c p) j -> p c j", p=128))
            emit_loads(3)
            # r0 staged on partition row 0 (rhs of K=1 fold matmuls)
            r01 = wp.tile([1, J], BF16, tag="r01")
            nc.sync.dma_start(
                out=r01, in_=bass.AP(tensor=r0_d, offset=0, ap=[[J, 1], [1, J]])
            )
            ones1 = wp.tile([1, 128], BF16, tag="ones1")
            nc.vector.memset(ones1, 1.0)
            id128 = wp.tile([128, 128], F32, tag="id128")
            from concourse.masks import make_identity

            make_identity(nc, id128)
            # bf16 identity produced by ACT (keeps transpose waits mergeable)
            id_bf = wp.tile([BSH, BSH], BF16, tag="id_bf")
            nc.scalar.activation(out=id_bf, in_=id128[:BSH, :BSH], func=AF.Copy)
            # PE warmup on id128 so later fp32 transposes never owe a Pool wait
            warm_ps = hp.tile([2, 2], F32, tag="h")
            nc.tensor.transpose(warm_ps, id128[0:2, 0:2], id128[0:2, 0:2])
            # fp16 -base^ rows staged in DRAM for the partition broadcast
            nb16_t = dramp.tile([BSH, J], F16, tag="nb16")

            # ================= setup: cond / q~ =================
            tpe_bf = wp.tile([BSH, D], BF16, tag="tpe_bf")
            nc.scalar.activation(out=tpe_bf, in_=tpe, func=AF.Copy)
            tpeT = wp.tile([128, DC, BSH], BF16, tag="tpeT")
            for c in range(DC):
                ps = hp.tile([128, BSH], BF16, tag="h")
                nc.tensor.transpose(ps, tpe_bf[:, c * 128:(c + 1) * 128], id_bf)
                nc.scalar.activation(out=tpeT[:, c, :], in_=ps, func=AF.Copy)
            # Z.T = Wt1 @ tpe.T (+bt1), silu = z * sigmoid(z)
            s_sb = wp.tile([128, DC, BSH], BF16, tag="s_sb")
            zl_sb = wp.tile([128, DC, BSH], F32, tag="zl_sb")
            sg_sb = wp.tile([128, DC, BSH], F32, tag="sg_sb")
            for zt in range(DC):
                ps = hp.tile([128, BSH], F32, tag="h")
                for c in range(DC):
                    nc.tensor.matmul(
                        ps, wt1t[:, c, zt * 128:(zt + 1) * 128], tpeT[:, c, :],
                        start=(c == 0), stop=(c == DC - 1),
                    )
                nc.scalar.activation(
                    out=zl_sb[:, zt, :], in_=ps, func=AF.Identity,
                    bias=bt1c[:, zt:zt + 1], scale=1.0,
                )
                nc.scalar.activation(
                    out=sg_sb[:, zt, :], in_=ps, func=AF.Sigmoid,
                    bias=bt1c[:, zt:zt + 1], scale=1.0,
                )
            nc.vector.tensor_mul(
                s_sb.rearrange("p c b -> p (c b)"),
                zl_sb.rearrange("p c b -> p (c b)"),
                sg_sb.rearrange("p c b -> p (c b)"),
            )
            # condT = Wt2 @ silu (+bt2)
            condT = wp.tile([128, DC, BSH], BF16, tag="condT")
            for ct in range(DC):
                ps = hp.tile([128, BSH], F32, tag="h")
                for c in range(DC):
                    nc.tensor.matmul(
                        ps, wt2t[:, c, ct * 128:(ct + 1) * 128], s_sb[:, c, :],
                        start=(c == 0), stop=(c == DC - 1),
                    )
                nc.scalar.activation(
                    out=condT[:, ct, :], in_=ps, func=AF.Identity,
                    bias=bt2c[:, ct:ct + 1], scale=1.0,
                )
            # qcT = qeT + condT ; q~T = M1.T @ qcT  (bf16)
            qcT = wp.tile([128, DC, BSH], BF16, tag="qcT")
            nc.vector.tensor_add(qcT[:], qet[:], condT[:])
            qtT = wp.tile([128, DC, BSH], BF16, tag="qtT")
            for mt in range(DC):
                ps = hp.tile([128, BSH], F32, tag="h")
                for c in range(DC):
                    nc.tensor.matmul(
                        ps, m1[:, c, mt * 128:(mt + 1) * 128], qcT[:, c, :],
                        start=(c == 0), stop=(c == DC - 1),
                    )
                nc.scalar.activation(out=qtT[:, mt, :], in_=ps, func=AF.Copy)

            # ws across all rows (read by batched base matmuls)
            ws_sb = wp.tile([128, DC, BSH], BF16, tag="ws_sb")

            # ============ skewed pipeline over batch rows ============
            def emit_attn(b):
                """scores (PE) -> exp/norm (ACT) -> w bounce."""
                s = st[b]
                emit_loads(b)
                te_t, xrow = s["te"], s["xrow"]
                scs = hp.tile([1, 2, 512], F32, tag="h", name=f"sc{b}")
                sc = [scs[:, 0, :], scs[:, 1, :]]
                for h in range(2):
                    for c in range(DC):
                        nc.tensor.matmul(
                            sc[h], qtT[:, c, b:b + 1],
                            te_t[:, c, h * 512:(h + 1) * 512],
                            start=(c == 0), stop=False,
                        )
                for h in range(2):
                    nc.tensor.matmul(
                        sc[h], ones1[0:1, 0:1],
                        xrow[0:1, h * 512:(h + 1) * 512],
                        start=False, stop=True,
                    )
                exp_row = rowp.tile([1, V], F32, tag="exp", name=f"exp{b}")
                se = [tinyp.tile([1, 1], F32, tag="t1", name=f"se{h}_{b}") for h in range(2)]
                for h in range(2):
                    nc.scalar.activation(
                        out=exp_row[:, h * 512:(h + 1) * 512], in_=sc[h],
                        func=AF.Exp, accum_out=se[h],
                    )
                sume = tinyp.tile([1, 1], F32, tag="t1", name=f"sume{b}")
                nc.vector.tensor_add(sume, se[0], se[1])
                rec = tinyp.tile([1, 1], F32, tag="t1", name=f"rec{b}")
                nc.vector.reciprocal(rec, sume)
                expn = rowp.tile([1, V], BF16, tag="expn", name=f"expn{b}")
                nc.scalar.activation(
                    out=expn, in_=exp_row, func=AF.Copy, bias=0.0, scale=rec[:, :1]
                )
                # w row out (also the DRAM bounce source for the broadcast)
                nc.sync.dma_start(out=wout_d[b:b + 1, :], in_=expn)
                ebc = ebp.tile([128, V], BF16, tag="ebc", name=f"ebc{b}")
                nc.sync.dma_start(
                    out=ebc,
                    in_=bass.AP(tensor=wout_d, offset=b * V, ap=[[0, 128], [1, V]]),
                )
                s["ebc"] = ebc

            def ws_chunk(b, c):
                """one DVE multiply+reduce chunk of ws row b; c==DC adds cond."""
                s = st[b]
                if c == DC:
                    nc.vector.tensor_add(
                        ws_sb[:, :, b:b + 1].rearrange("p c one -> p (c one)"),
                        s["ws2"], condT[:, :, b],
                    )
                    return
                if "ws2" not in s:
                    s["ws2"] = tinyp.tile([128, DC], F32, tag="ws2", name=f"ws2_{b}")
                wscr = scrp.tile([128, V], BF16, tag="wscr")
                nc.vector.scalar_tensor_tensor(
                    out=wscr, in0=s["te"][:, c, :], scalar=0.0, in1=s["ebc"],
                    op0=ALU.bypass, op1=ALU.mult,
                    accum_out=s["ws2"][:, c:c + 1],
                )

            def emit_ws(b):
                for c in range(DC + 1):
                    ws_chunk(b, c)

            def emit_base(lo, hi):
                """batched base matmul for rows lo..hi; -base^ -> DRAM."""
                n = hi - lo + 1
                bp_ps = hp.tile([4, J], F32, tag="h", name=f"base{lo}")
                for h in range(2):
                    for c in range(DC):
                        nc.tensor.matmul(
                            bp_ps[:n, h * 512:(h + 1) * 512],
                            ws_sb[:, c, lo:hi + 1],
                            m3[:, c, h * 512:(h + 1) * 512],
                            start=(c == 0), stop=False,
                        )
                for h in range(2):
                    nc.tensor.matmul(
                        bp_ps[:n, h * 512:(h + 1) * 512],
                        ones1[0:1, 0:n], r01[0:1, h * 512:(h + 1) * 512],
                        start=False, stop=True,
                    )
                negb16 = wp.tile([4, J], F16, tag="negb16", name=f"negb16_{lo}")
                nc.scalar.activation(
                    out=negb16[:n], in_=bp_ps[:n], func=AF.Copy, bias=0.0, scale=-1.0
                )
                nc.sync.dma_start(out=nb16_t[lo:hi + 1, :], in_=negb16[:n])
                negb = wp.tile([4, J], F32, tag="negb", name=f"negb{lo}")
                nc.scalar.activation(
                    out=negb[:n], in_=bp_ps[:n], func=AF.Copy, bias=0.0, scale=-1.0
                )
                nc.sync.dma_start(out=nb_d[lo:hi + 1, :], in_=negb[:n])

            def emit_decode(b, ws_row=None):
                """H matmuls (PE) + sign-split max/accum drains (DVE), with the
                ws chunks of row ws_row threaded between drain pairs so they
                never block a full row of drains."""
                s = st[b]
                te_t = s["te"]
                nbc = nbp.tile([128, J], F16, tag="nbc", name=f"nbc{b}")
                nc.sync.dma_start(
                    out=nbc,
                    in_=bass.AP(tensor=nb16_t.tensor, offset=nb16_t.offset + b * J,
                                ap=[[0, 128], [1, J]]),
                )
                ppos = tinyp.tile([128, VT], F32, tag="ppos", name=f"ppos{b}")
                pneg = tinyp.tile([128, VT], F32, tag="pneg", name=f"pneg{b}")
                for vt in range(VT):
                    t = hp.tile([128, 2, 512], F32, tag="h", name=f"h{b}_{vt}")
                    for c in range(DC):
                        for h in range(2):
                            nc.tensor.matmul(
                                t[:, h, :],
                                te_t[:, c, vt * 128:(vt + 1) * 128],
                                bm[:, c, h * 512:(h + 1) * 512],
                                start=(c == 0), stop=(c == DC - 1),
                            )
                    tf = t.rearrange("p a v -> p (a v)")
                    nc.vector.scalar_tensor_tensor(
                        out=tf[:, :jp], in0=tf[:, :jp], scalar=0.0,
                        in1=nbc[:, :jp], op0=ALU.bypass, op1=ALU.max,
                        accum_out=ppos[:, vt:vt + 1],
                    )
                    nc.vector.scalar_tensor_tensor(
                        out=tf[:, jp:], in0=tf[:, jp:], scalar=0.0,
                        in1=nbc[:, jp:], op0=ALU.bypass, op1=ALU.max,
                        accum_out=pneg[:, vt:vt + 1],
                    )
                    if ws_row is not None and vt < DC:
                        ws_chunk(ws_row, vt)
                        if vt == DC - 1:
                            ws_chunk(ws_row, DC)
                nc.sync.dma_start(out=pp_d[b, 0], in_=ppos)
                nc.sync.dma_start(out=pp_d[b, 1], in_=pneg)

            for i in range(N_ITER):
                if i + 2 < BSH:
                    emit_loads(i + 2)
                if i < BSH:
                    emit_attn(i)
                b = DEC_AT.get(i)
                if b is None:
                    if i < BSH:
                        emit_ws(i)
                else:
                    emit_decode(b, ws_row=(i if i < BSH else None))
                    st[b].clear()
                for (lo, hi) in GROUPS:
                    if hi == i:
                        emit_base(lo, hi)

    return nc


_NC_CACHE: dict = {}


def _get_nc(jp: int) -> bass.Bass:
    if jp not in _NC_CACHE:
        nc = build_nc(jp)
        nc.finalize()
        _NC_CACHE[jp] = nc
    return _NC_CACHE[jp]


def _pos_encoding() -> np.ndarray:
    pos = np.arange(MAX_LEN, dtype=np.float32)[:, None]
    div = np.exp(np.arange(0, D, 2, dtype=np.float32) * (-np.log(10000.0) / D))
    pe = np.zeros((MAX_LEN, D), dtype=np.float32)
    pe[:, 0::2] = np.sin(pos * div)
    pe[:, 1::2] = np.cos(pos * div)
    return pe


def prepare_in_maps(inputs: dict):
    f32 = lambda a: np.ascontiguousarray(np.asarray(a), dtype=np.float32)
    bf = lambda a: np.ascontiguousarray(np.asarray(a, dtype=np.float32).astype(BF_NP))
    x = np.asarray(inputs["x"], dtype=np.float32)
    ts = np.ascontiguousarray(np.asarray(inputs["timesteps"]).astype(np.int32).reshape(B, 1))
    qe = np.asarray(inputs["query_emb"], dtype=np.float32)
    te = np.asarray(inputs["target_emb"], dtype=np.float32)
    Wq, Wk, Wv, Wp = (f32(inputs[k]) for k in ("Wq", "Wk", "Wv", "Wp"))
    bp = f32(inputs["bp"])
    Wt1, bt1, Wt2, bt2 = (f32(inputs[k]) for k in ("Wt1", "bt1", "Wt2", "bt2"))
    Wd1, bd1, Wd2, bd2 = (f32(inputs[k]) for k in ("Wd1", "bd1", "Wd2", "bd2"))

    pe = _pos_encoding()
    M1 = Wq.T @ Wk
    A = np.ascontiguousarray(Wd1[:, :D].T)
    Bm = Wd1[:, D:].T
    M3 = (Wv.T @ Wp.T) @ A
    r0 = bp @ A + bd1
    w2 = Wd2[0].copy()
    bd2_val = float(bd2.reshape(-1)[0])
    bt1c = np.ascontiguousarray(bt1.reshape(DC, 128).T)
    bt2c = np.ascontiguousarray(bt2.reshape(DC, 128).T)

    # sign permutation: positive-w2 columns first; |w2| folded into the
    # j-indexed tensors so the decoder reduction is a plain +/- sum.  jp is
    # kept even (DVE 2x alignment) by zeroing the smallest-|w2| positive
    # column and placing it in the negative block (contributes exactly 0;
    # its true contribution is ~|w2|min * 0.5 ~ 1e-5, far below tolerance).
    pos_idx = np.where(w2 >= 0)[0]
    neg_idx = np.where(w2 < 0)[0]
    if len(pos_idx) % 2 == 1:
        drop = pos_idx[np.argmin(np.abs(w2[pos_idx]))]
        w2[drop] = 0.0
        pos_idx = pos_idx[pos_idx != drop]
        neg_idx = np.concatenate([neg_idx, [drop]])
    perm = np.concatenate([pos_idx, neg_idx])
    jp = int(len(pos_idx))
    aw = np.abs(w2)[perm]
    Bmh = np.ascontiguousarray(Bm[:, perm] * aw)
    M3h = np.ascontiguousarray(M3[:, perm] * aw)
    r0h = np.ascontiguousarray(r0[perm] * aw)

    shared = dict(
        pe=pe, wt1t=bf(Wt1.T), wt2t=bf(Wt2.T), bt1c=bt1c, bt2c=bt2c,
        m1=bf(M1), m3=bf(M3h), bm=bf(Bmh), r0=bf(r0h),
    )
    in_maps = []
    for i in range(NCORES):
        s = slice(i * BSH, (i + 1) * BSH)
        in_maps.append(
            dict(
                te=bf(te[s].transpose(0, 2, 1)),
                x=bf(x[s]),
                ts=np.ascontiguousarray(ts[s]),
                qet=bf(qe[s].T),
                **shared,
            )
        )
    return in_maps, jp, bd2_val


def assemble(results: list, jp: int, bd2_val: float) -> np.ndarray:
    """Host-side final assembly from per-core device outputs."""
    outs = []
    for r in results:
        pp = np.asarray(r["pp"], dtype=np.float32).reshape(BSH, 2, 128, VT)
        nb = np.asarray(r["nb"], dtype=np.float32).reshape(BSH, J)  # -base^
        w = np.asarray(r["wout"]).astype(np.float32).reshape(BSH, V)
        C = -(nb[:, :jp].sum(axis=1) - nb[:, jp:].sum(axis=1))  # [BSH]
        pc = pp[:, 0] - pp[:, 1]                                # [BSH, 128, VT]
        p = pc.transpose(0, 2, 1).reshape(BSH, V)               # v = vt*128 + part
        outs.append(p + C[:, None] + bd2_val + w)
    return np.concatenate(outs, axis=0).astype(np.float32)


def run(inputs: dict, trace: bool = False):
    in_maps, jp, bd2_val = prepare_in_maps(inputs)
    nc = _get_nc(jp)
    res = run_bass_kernel_spmd(nc, in_maps, list(range(NCORES)), trace=trace)
    out = assemble(res.results, jp, bd2_val)
    return out, res


def kernel(**inputs) -> np.ndarray:
    out, _ = run(inputs, trace=False)
    return out


# revision 17
# speedup vs baseline: 1.1540x; 1.1001x over previous
"""Trainium2 Bass kernel for the single-query-attention diffusion decoder.

Full-input contract: kernel(**inputs) -> np.ndarray [B, V].
Data-parallel over batch across 8 NeuronCores (16 rows each).

Math (reference restructured):
    cond  = silu(pe[t] @ Wt1.T + bt1) @ Wt2.T + bt2            [B, D]
    q~    = (query + cond) @ M1,  M1 = Wq.T @ Wk               [B, D]
    s[v]  = q~ . T[v] + x[v]   (+ q~.cond, dropped: softmax shift-invariant)
    w     = softmax(s)
    ws    = sum_v w[v] T[v] + cond                             [D]
    base  = ws @ M3 + r0,  M3 = Wv.T @ Wp.T @ Wd1[:, :D].T,
            r0 = bp @ Wd1[:, :D].T + bd1
    p[v]  = sum_j w2[j] relu(T[v] @ Bm + base)[j] + bd2 + w[v]
            Bm = Wd1[:, D:].T,  w2 = Wd2[0]

Decoder sign trick: with |w2| folded into Bm/M3/r0 (columns scaled) and the
j axis permuted so sign(w2) = +1 columns come first (jp kept even so the DVE
slices stay 4B-aligned),
    w2[j] relu(h + base)[j] = sgn_j * (max(h^, -base^) + base^)_j
so p[v] = [sum_{j<jp} - sum_{j>=jp}] max(h^_jv, -base^_j)  + C + bd2 + w[v]
with C = sum_j sgn_j base^_j.  Each H PSUM tile is drained by two fused
tensor_tensor_reduce ops (max + add-reduce in one DVE instruction) -- no PE
base-fold matmuls, no separate accumulator reads.  base is one batched PE
matmul per row group; -base^ bounces through DRAM into an fp16
partition-broadcast SBUF tile.  ppos/pneg/negbase/w are DMA'd out and the
final p (+C +bd2 +w) is assembled on host.
"""

import os
import sys

for _p in ("/opt/trn_rl_repo", "/opt/trn_rl_repo/concourse"):
    if os.path.isdir(_p) and _p not in sys.path:
        sys.path.append(_p)

import numpy as np
import ml_dtypes

import concourse.bass as bass
import concourse.tile as tile
from concourse import bacc, mybir
from concourse.bass_utils import run_bass_kernel_spmd

F32 = mybir.dt.float32
F16 = mybir.dt.float16
BF16 = mybir.dt.bfloat16
I32 = mybir.dt.int32
AF = mybir.ActivationFunctionType
ALU = mybir.AluOpType
BF_NP = ml_dtypes.bfloat16

NCORES = 8
B = 128
BSH = B // NCORES  # 16 batch rows per core
D = 512
V = 1024
J = 2 * D  # 1024 decoder hidden
DC = D // 128  # 4 d-chunks
VT = V // 128  # 8 v-tiles
MAX_LEN = 5000

# base-matmul row groups: pairs, emitted at iteration 2k+1 right after the
# interleaved ws chunks of row 2k+1 complete; decode(b) runs at iteration
# b+2 so its group (emitted at iteration b|1 <= b+1) is always ready.
GROUPS = [(2 * k, 2 * k + 1) for k in range(BSH // 2)]
# iteration -> decoded row (attention row i is emitted at iteration i)
DEC_AT = {i: i - 2 for i in range(2, 18)}
N_ITER = 18


def build_nc(jp: int) -> bass.Bass:
    # Bacc (not plain Bass): its finalize() legalizes sync waits
    # (generate_event_semaphores) to TRN2's 1-wait-per-instruction limit.
    nc = bacc.Bacc()

    # ---- per-core inputs ----
    te_d = nc.declare_dram_parameter("te", [BSH, D, V], BF16, isOutput=False)
    x_d = nc.declare_dram_parameter("x", [BSH, V], BF16, isOutput=False)
    ts_d = nc.declare_dram_parameter("ts", [BSH, 1], I32, isOutput=False)
    qet_d = nc.declare_dram_parameter("qet", [D, BSH], BF16, isOutput=False)
    # ---- replicated (host-folded) weights ----
    pe_d = nc.declare_dram_parameter("pe", [MAX_LEN, D], F32, isOutput=False)
    wt1t_d = nc.declare_dram_parameter("wt1t", [D, D], BF16, isOutput=False)
    wt2t_d = nc.declare_dram_parameter("wt2t", [D, D], BF16, isOutput=False)
    bt1c_d = nc.declare_dram_parameter("bt1c", [128, DC], F32, isOutput=False)
    bt2c_d = nc.declare_dram_parameter("bt2c", [128, DC], F32, isOutput=False)
    m1_d = nc.declare_dram_parameter("m1", [D, D], BF16, isOutput=False)
    m3_d = nc.declare_dram_parameter("m3", [D, J], BF16, isOutput=False)
    bm_d = nc.declare_dram_parameter("bm", [D, J], BF16, isOutput=False)
    r0_d = nc.declare_dram_parameter("r0", [J], BF16, isOutput=False)
    # ---- outputs (host assembles p from these) ----
    nb_d = nc.declare_dram_parameter("nb", [BSH, J], F32, isOutput=True)
    wout_d = nc.declare_dram_parameter("wout", [BSH, V], BF16, isOutput=True)
    pp_d = nc.declare_dram_parameter("pp", [BSH, 2, 128, VT], F32, isOutput=True)

    with tile.TileContext(nc) as tc:
        with (
            tc.tile_pool(name="w", bufs=1) as wp,
            tc.tile_pool(name="te", bufs=BSH) as tep,
            tc.tile_pool(name="rows", bufs=3) as rowp,
            tc.tile_pool(name="nbc", bufs=2) as nbp,
            tc.tile_pool(name="ebc", bufs=2) as ebp,
            tc.tile_pool(name="scr", bufs=1) as scrp,
            tc.tile_pool(name="tiny", bufs=8) as tinyp,
            tc.tile_pool(name="dramp", bufs=1, space="DRAM") as dramp,
            tc.tile_pool(name="hp", bufs=3, space="PSUM") as hp,
            tc.tile_pool(name="scp", bufs=1, space="PSUM") as scp,
        ):
            st = [dict() for _ in range(BSH)]

            def emit_loads(b):
                s = st[b]
                if "te" in s:
                    return
                s["xrow"] = rowp.tile([1, V], BF16, tag="xrow", name=f"xrow{b}")
                nc.sync.dma_start(out=s["xrow"], in_=x_d[b:b + 1, :])
                s["te"] = tep.tile([128, DC, V], BF16, tag="te", name=f"te{b}")
                nc.sync.dma_start(
                    out=s["te"], in_=te_d[b].rearrange("(c p) v -> p c v", p=128)
                )

            # ====== loads, ordered so the pipeline starts ASAP ======
            ts_sb = wp.tile([BSH, 1], I32, tag="ts")
            nc.sync.dma_start(out=ts_sb, in_=ts_d[:])
            # dummy 1-row gather warms the Q7 indirect-DMA kernel (~6us IRAM
            # load) while the ts/weight DMAs stream in
            zidx = wp.tile([2, 1], I32, tag="zidx")
            nc.vector.memset(zidx, 0)
            warm_g = wp.tile([2, D], F32, tag="warm_g")
            nc.gpsimd.indirect_dma_start(
                out=warm_g[:],
                out_offset=None,
                in_=pe_d[:],
                in_offset=bass.IndirectOffsetOnAxis(ap=zidx[:, :1], axis=0),
            )
            # gather pe rows by timestep (gpsimd queue; only needs ts)
            tpe = wp.tile([BSH, D], F32, tag="tpe")
            nc.gpsimd.indirect_dma_start(
                out=tpe[:],
                out_offset=None,
                in_=pe_d[:],
                in_offset=bass.IndirectOffsetOnAxis(ap=ts_sb[:, :1], axis=0),
            )
            wt1t = wp.tile([128, DC, D], BF16, tag="wt1t")
            nc.sync.dma_start(out=wt1t, in_=wt1t_d[:].rearrange("(c p) z -> p c z", p=128))
            emit_loads(0)
            bt1c = wp.tile([128, DC], F32, tag="bt1c")
            nc.sync.dma_start(out=bt1c, in_=bt1c_d[:])
            bt2c = wp.tile([128, DC], F32, tag="bt2c")
            nc.sync.dma_start(out=bt2c, in_=bt2c_d[:])
            wt2t = wp.tile([128, DC, D], BF16, tag="wt2t")
            nc.sync.dma_start(out=wt2t, in_=wt2t_d[:].rearrange("(c p) z -> p c z", p=128))
            emit_loads(1)
            qet = wp.tile([128, DC, BSH], BF16, tag="qet")
            nc.sync.dma_start(out=qet, in_=qet_d[:].rearrange("(c p) b -> p c b", p=128))
            m1 = wp.tile([128, DC, D], BF16, tag="m1")
            nc.sync.dma_start(out=m1, in_=m1_d[:].rearrange("(c p) z -> p c z", p=128))
            emit_loads(2)
            m3 = wp.tile([128, DC, J], BF16, tag="m3")
            nc.sync.dma_start(out=m3, in_=m3_d[:].rearrange("(c p) j -> p c j", p=128))
            bm = wp.tile([128, DC, J], BF16, tag="bm")
            nc.sync.dma_start(out=bm, in_=bm_d[:].rearrange("(c p) j -> p c j", p=128))
            emit_loads(3)
            # r0 staged on partition row 0 (rhs of K=1 fold matmuls)
            r01 = wp.tile([1, J], BF16, tag="r01")
            nc.sync.dma_start(
                out=r01, in_=bass.AP(tensor=r0_d, offset=0, ap=[[J, 1], [1, J]])
            )
            ones1 = wp.tile([1, 128], BF16, tag="ones1")
            nc.vector.memset(ones1, 1.0)
            id128 = wp.tile([128, 128], F32, tag="id128")
            from concourse.masks import make_identity

            make_identity(nc, id128)
            # bf16 identity produced by ACT (keeps transpose waits mergeable)
            id_bf = wp.tile([BSH, BSH], BF16, tag="id_bf")
            nc.scalar.activation(out=id_bf, in_=id128[:BSH, :BSH], func=AF.Copy)
            # PE warmup on id128 so later fp32 transposes never owe a Pool wait
            warm_ps = scp.tile([2, 2], F32, tag="sc")
            nc.tensor.transpose(warm_ps, id128[0:2, 0:2], id128[0:2, 0:2])
            # fp16 -base^ rows staged in DRAM for the partition broadcast
            nb16_t = dramp.tile([BSH, J], F16, tag="nb16")

            # ================= setup: cond / q~ =================
            tpe_bf = wp.tile([BSH, D], BF16, tag="tpe_bf")
            nc.scalar.activation(out=tpe_bf, in_=tpe, func=AF.Copy)
            tpeT = wp.tile([128, DC, BSH], BF16, tag="tpeT")
            for c in range(DC):
                ps = scp.tile([128, BSH], BF16, tag="sc")
                nc.tensor.transpose(ps, tpe_bf[:, c * 128:(c + 1) * 128], id_bf)
                nc.scalar.activation(out=tpeT[:, c, :], in_=ps, func=AF.Copy)
            # Z.T = Wt1 @ tpe.T (+bt1), silu = z * sigmoid(z)
            s_sb = wp.tile([128, DC, BSH], BF16, tag="s_sb")
            zl_sb = wp.tile([128, DC, BSH], F32, tag="zl_sb")
            sg_sb = wp.tile([128, DC, BSH], F32, tag="sg_sb")
            for zt in range(DC):
                ps = scp.tile([128, BSH], F32, tag="sc")
                for c in range(DC):
                    nc.tensor.matmul(
                        ps, wt1t[:, c, zt * 128:(zt + 1) * 128], tpeT[:, c, :],
                        start=(c == 0), stop=(c == DC - 1),
                    )
                nc.scalar.activation(
                    out=zl_sb[:, zt, :], in_=ps, func=AF.Identity,
                    bias=bt1c[:, zt:zt + 1], scale=1.0,
                )
                nc.scalar.activation(
                    out=sg_sb[:, zt, :], in_=ps, func=AF.Sigmoid,
                    bias=bt1c[:, zt:zt + 1], scale=1.0,
                )
            nc.vector.tensor_mul(
                s_sb.rearrange("p c b -> p (c b)"),
                zl_sb.rearrange("p c b -> p (c b)"),
                sg_sb.rearrange("p c b -> p (c b)"),
            )
            # condT = Wt2 @ silu (+bt2)
            condT = wp.tile([128, DC, BSH], BF16, tag="condT")
            for ct in range(DC):
                ps = scp.tile([128, BSH], F32, tag="sc")
                for c in range(DC):
                    nc.tensor.matmul(
                        ps, wt2t[:, c, ct * 128:(ct + 1) * 128], s_sb[:, c, :],
                        start=(c == 0), stop=(c == DC - 1),
                    )
                nc.scalar.activation(
                    out=condT[:, ct, :], in_=ps, func=AF.Identity,
                    bias=bt2c[:, ct:ct + 1], scale=1.0,
                )
            # qcT = qeT + condT ; q~T = M1.T @ qcT  (bf16)
            qcT = wp.tile([128, DC, BSH], BF16, tag="qcT")
            nc.vector.tensor_add(qcT[:], qet[:], condT[:])
            qtT = wp.tile([128, DC, BSH], BF16, tag="qtT")
            for mt in range(DC):
                ps = scp.tile([128, BSH], F32, tag="sc")
                for c in range(DC):
                    nc.tensor.matmul(
                        ps, m1[:, c, mt * 128:(mt + 1) * 128], qcT[:, c, :],
                        start=(c == 0), stop=(c == DC - 1),
                    )
                nc.scalar.activation(out=qtT[:, mt, :], in_=ps, func=AF.Copy)

            # ws across all rows (read by batched base matmuls)
            ws_sb = wp.tile([128, DC, BSH], BF16, tag="ws_sb")

            # ============ skewed pipeline over batch rows ============
            def emit_attn(b):
                """scores (PE) -> exp/norm (ACT) -> w bounce."""
                s = st[b]
                emit_loads(b)
                te_t, xrow = s["te"], s["xrow"]
                scs = scp.tile([1, 2, 512], F32, tag="sc", name=f"sc{b}")
                sc = [scs[:, 0, :], scs[:, 1, :]]
                for h in range(2):
                    for c in range(DC):
                        nc.tensor.matmul(
                            sc[h], qtT[:, c, b:b + 1],
                            te_t[:, c, h * 512:(h + 1) * 512],
                            start=(c == 0), stop=False,
                        )
                for h in range(2):
                    nc.tensor.matmul(
                        sc[h], ones1[0:1, 0:1],
                        xrow[0:1, h * 512:(h + 1) * 512],
                        start=False, stop=True,
                    )
                exp_row = rowp.tile([1, V], F32, tag="exp", name=f"exp{b}")
                se = [tinyp.tile([1, 1], F32, tag="t1", name=f"se{h}_{b}") for h in range(2)]
                for h in range(2):
                    nc.scalar.activation(
                        out=exp_row[:, h * 512:(h + 1) * 512], in_=sc[h],
                        func=AF.Exp, accum_out=se[h],
                    )
                sume = tinyp.tile([1, 1], F32, tag="t1", name=f"sume{b}")
                nc.vector.tensor_add(sume, se[0], se[1])
                rec = tinyp.tile([1, 1], F32, tag="t1", name=f"rec{b}")
                nc.vector.reciprocal(rec, sume)
                expn = rowp.tile([1, V], BF16, tag="expn", name=f"expn{b}")
                nc.scalar.activation(
                    out=expn, in_=exp_row, func=AF.Copy, bias=0.0, scale=rec[:, :1]
                )
                # w row out (also the DRAM bounce source for the broadcast)
                nc.sync.dma_start(out=wout_d[b:b + 1, :], in_=expn)
                ebc = ebp.tile([128, V], BF16, tag="ebc", name=f"ebc{b}")
                nc.sync.dma_start(
                    out=ebc,
                    in_=bass.AP(tensor=wout_d, offset=b * V, ap=[[0, 128], [1, V]]),
                )
                s["ebc"] = ebc

            def ws_chunk(b, c):
                """one DVE multiply+reduce chunk of ws row b; c==DC adds cond."""
                s = st[b]
                if c == DC:
                    nc.vector.tensor_add(
                        ws_sb[:, :, b:b + 1].rearrange("p c one -> p (c one)"),
                        s["ws2"], condT[:, :, b],
                    )
                    return
                if "ws2" not in s:
                    s["ws2"] = tinyp.tile([128, DC], F32, tag="ws2", name=f"ws2_{b}")
                wscr = scrp.tile([128, V], BF16, tag="wscr")
                nc.vector.scalar_tensor_tensor(
                    out=wscr, in0=s["te"][:, c, :], scalar=0.0, in1=s["ebc"],
                    op0=ALU.bypass, op1=ALU.mult,
                    accum_out=s["ws2"][:, c:c + 1],
                )

            def emit_ws(b):
                for c in range(DC + 1):
                    ws_chunk(b, c)

            def emit_base(lo, hi):
                """batched base matmul for rows lo..hi; -base^ -> DRAM."""
                n = hi - lo + 1
                bp_ps = hp.tile([4, J], F32, tag="h", name=f"base{lo}")
                for h in range(2):
                    for c in range(DC):
                        nc.tensor.matmul(
                            bp_ps[:n, h * 512:(h + 1) * 512],
                            ws_sb[:, c, lo:hi + 1],
                            m3[:, c, h * 512:(h + 1) * 512],
                            start=(c == 0), stop=False,
                        )
                for h in range(2):
                    nc.tensor.matmul(
                        bp_ps[:n, h * 512:(h + 1) * 512],
                        ones1[0:1, 0:n], r01[0:1, h * 512:(h + 1) * 512],
                        start=False, stop=True,
                    )
                negb16 = wp.tile([4, J], F16, tag="negb16", name=f"negb16_{lo}")
                nc.scalar.activation(
                    out=negb16[:n], in_=bp_ps[:n], func=AF.Copy, bias=0.0, scale=-1.0
                )
                nc.sync.dma_start(out=nb16_t[lo:hi + 1, :], in_=negb16[:n])
                negb = wp.tile([4, J], F32, tag="negb", name=f"negb{lo}")
                nc.scalar.activation(
                    out=negb[:n], in_=bp_ps[:n], func=AF.Copy, bias=0.0, scale=-1.0
                )
                nc.sync.dma_start(out=nb_d[lo:hi + 1, :], in_=negb[:n])

            def emit_decode(b, ws_row=None):
                """H matmuls (PE) + sign-split max/accum drains (DVE), with the
                ws chunks of row ws_row threaded between drain pairs so they
                never block a full row of drains."""
                s = st[b]
                te_t = s["te"]
                nbc = nbp.tile([128, J], F16, tag="nbc", name=f"nbc{b}")
                nc.sync.dma_start(
                    out=nbc,
                    in_=bass.AP(tensor=nb16_t.tensor, offset=nb16_t.offset + b * J,
                                ap=[[0, 128], [1, J]]),
                )
                ppos = tinyp.tile([128, VT], F32, tag="ppos", name=f"ppos{b}")
                pneg = tinyp.tile([128, VT], F32, tag="pneg", name=f"pneg{b}")
                for vt in range(VT):
                    t = hp.tile([128, 2, 512], F32, tag="h", name=f"h{b}_{vt}")
                    for c in range(DC):
                        for h in range(2):
                            nc.tensor.matmul(
                                t[:, h, :],
                                te_t[:, c, vt * 128:(vt + 1) * 128],
                                bm[:, c, h * 512:(h + 1) * 512],
                                start=(c == 0), stop=(c == DC - 1),
                            )
                    tf = t.rearrange("p a v -> p (a v)")
                    nc.vector.scalar_tensor_tensor(
                        out=tf[:, :jp], in0=tf[:, :jp], scalar=0.0,
                        in1=nbc[:, :jp], op0=ALU.bypass, op1=ALU.max,
                        accum_out=ppos[:, vt:vt + 1],
                    )
                    nc.vector.scalar_tensor_tensor(
                        out=tf[:, jp:], in0=tf[:, jp:], scalar=0.0,
                        in1=nbc[:, jp:], op0=ALU.bypass, op1=ALU.max,
                        accum_out=pneg[:, vt:vt + 1],
                    )
                    if ws_row is not None and vt < DC:
                        ws_chunk(ws_row, vt)
                        if vt == DC - 1:
                            ws_chunk(ws_row, DC)
                nc.sync.dma_start(out=pp_d[b, 0], in_=ppos)
                nc.sync.dma_start(out=pp_d[b, 1], in_=pneg)

            for i in range(N_ITER):
                if i + 2 < BSH:
                    emit_loads(i + 2)
                if i < BSH:
                    emit_attn(i)
                b = DEC_AT.get(i)
                if b is None:
                    if i < BSH:
                        emit_ws(i)
                else:
                    emit_decode(b, ws_row=(i if i < BSH else None))
                    st[b].clear()
                for (lo, hi) in GROUPS:
                    if hi == i:
                        emit_base(lo, hi)

    return nc


_NC_CACHE: dict = {}


def _get_nc(jp: int) -> bass.Bass:
    if jp not in _NC_CACHE:
        nc = build_nc(jp)
        nc.finalize()
        _NC_CACHE[jp] = nc
    return _NC_CACHE[jp]


def _pos_encoding() -> np.ndarray:
    pos = np.arange(MAX_LEN, dtype=np.float32)[:, None]
    div = np.exp(np.arange(0, D, 2, dtype=np.float32) * (-np.log(10000.0) / D))
    pe = np.zeros((MAX_LEN, D), dtype=np.float32)
    pe[:, 0::2] = np.sin(pos * div)
    pe[:, 1::2] = np.cos(pos * div)
    return pe


def prepare_in_maps(inputs: dict):
    f32 = lambda a: np.ascontiguousarray(np.asarray(a), dtype=np.float32)
    bf = lambda a: np.ascontiguousarray(np.asarray(a, dtype=np.float32).astype(BF_NP))
    x = np.asarray(inputs["x"], dtype=np.float32)
    ts = np.ascontiguousarray(np.asarray(inputs["timesteps"]).astype(np.int32).reshape(B, 1))
    qe = np.asarray(inputs["query_emb"], dtype=np.float32)
    te = np.asarray(inputs["target_emb"], dtype=np.float32)
    Wq, Wk, Wv, Wp = (f32(inputs[k]) for k in ("Wq", "Wk", "Wv", "Wp"))
    bp = f32(inputs["bp"])
    Wt1, bt1, Wt2, bt2 = (f32(inputs[k]) for k in ("Wt1", "bt1", "Wt2", "bt2"))
    Wd1, bd1, Wd2, bd2 = (f32(inputs[k]) for k in ("Wd1", "bd1", "Wd2", "bd2"))

    pe = _pos_encoding()
    M1 = Wq.T @ Wk
    A = np.ascontiguousarray(Wd1[:, :D].T)
    Bm = Wd1[:, D:].T
    M3 = (Wv.T @ Wp.T) @ A
    r0 = bp @ A + bd1
    w2 = Wd2[0].copy()
    bd2_val = float(bd2.reshape(-1)[0])
    bt1c = np.ascontiguousarray(bt1.reshape(DC, 128).T)
    bt2c = np.ascontiguousarray(bt2.reshape(DC, 128).T)

    # sign permutation: positive-w2 columns first; |w2| folded into the
    # j-indexed tensors so the decoder reduction is a plain +/- sum.  jp is
    # kept even (DVE 2x alignment) by zeroing the smallest-|w2| positive
    # column and placing it in the negative block (contributes exactly 0;
    # its true contribution is ~|w2|min * 0.5 ~ 1e-5, far below tolerance).
    pos_idx = np.where(w2 >= 0)[0]
    neg_idx = np.where(w2 < 0)[0]
    if len(pos_idx) % 2 == 1:
        drop = pos_idx[np.argmin(np.abs(w2[pos_idx]))]
        w2[drop] = 0.0
        pos_idx = pos_idx[pos_idx != drop]
        neg_idx = np.concatenate([neg_idx, [drop]])
    perm = np.concatenate([pos_idx, neg_idx])
    jp = int(len(pos_idx))
    aw = np.abs(w2)[perm]
    Bmh = np.ascontiguousarray(Bm[:, perm] * aw)
    M3h = np.ascontiguousarray(M3[:, perm] * aw)
    r0h = np.ascontiguousarray(r0[perm] * aw)

    shared = dict(
        pe=pe, wt1t=bf(Wt1.T), wt2t=bf(Wt2.T), bt1c=bt1c, bt2c=bt2c,
        m1=bf(M1), m3=bf(M3h), bm=bf(Bmh), r0=bf(r0h),
    )
    in_maps = []
    for i in range(NCORES):
        s = slice(i * BSH, (i + 1) * BSH)
        in_maps.append(
            dict(
                te=bf(te[s].transpose(0, 2, 1)),
                x=bf(x[s]),
                ts=np.ascontiguousarray(ts[s]),
                qet=bf(qe[s].T),
                **shared,
            )
        )
    return in_maps, jp, bd2_val


def assemble(results: list, jp: int, bd2_val: float) -> np.ndarray:
    """Host-side final assembly from per-core device outputs."""
    outs = []
    for r in results:
        pp = np.asarray(r["pp"], dtype=np.float32).reshape(BSH, 2, 128, VT)
        nb = np.asarray(r["nb"], dtype=np.float32).reshape(BSH, J)  # -base^
        w = np.asarray(r["wout"]).astype(np.float32).reshape(BSH, V)
        C = -(nb[:, :jp].sum(axis=1) - nb[:, jp:].sum(axis=1))  # [BSH]
        pc = pp[:, 0] - pp[:, 1]                                # [BSH, 128, VT]
        p = pc.transpose(0, 2, 1).reshape(BSH, V)               # v = vt*128 + part
        outs.append(p + C[:, None] + bd2_val + w)
    return np.concatenate(outs, axis=0).astype(np.float32)


def run(inputs: dict, trace: bool = False):
    in_maps, jp, bd2_val = prepare_in_maps(inputs)
    nc = _get_nc(jp)
    res = run_bass_kernel_spmd(nc, in_maps, list(range(NCORES)), trace=trace)
    out = assemble(res.results, jp, bd2_val)
    return out, res


def kernel(**inputs) -> np.ndarray:
    out, _ = run(inputs, trace=False)
    return out
